# revision 22
# baseline (speedup 1.0000x reference)
"""BatchedLightSimulation Trainium2 kernel.

Math: the two causal convolutions (scintillation 990 taps, SiPM impulse 990
taps) compose into one 1979-tap causal filter c.  Folding the sum-by-16
downsample in gives

    out[row, s] = sum_delta c16[delta] * ug[row, 16*s + delta]

with c16[delta] = sum_{k=max(0,delta)}^{15} c[k - delta] and
ug[row, t] = gain[row] * u[row, t] (gain folded into the input on the
host).  c decays like exp(-l/15.3) so c16 truncated to delta >= -240 is
exact at fp32 precision.

Precision: x in single bf16 (no hi/lo split), W in fp8 e3m4 scaled by
16 (exact power of two, compensated by shipping x/16 -- bf16 absorbs it
in the exponent, so no epilogue scaling).  Measured end-to-end vs the
fp64 reference on the actual inputs: absmax-rel 4.34e-3, per-element
max 7.7e-3 -- inside the 2e-2 gate with >2.5x margin on either formula.
(x in fp8 was rejected: e3m4 x measures 9.4e-3 absmax / 1.7e-2
per-element -- too close to the gate.)

Device mapping (per core, 4 ninputs = 192 (n,d) rows):
  polyphase m = 16q + r; s tiled 4x100, tiles paired (p=0: s-tiles 0,1;
  p=1: s-tiles 2,3).  The banded weight is Toeplitz in (q_rel, s_rel), so
  ONE [128,128] weight slice per r serves every s-tile, and a pair's two
  tiles ride one matmul in the free dim:
      W_r.T[128q, 128s] @ X[q, p, r, :, :]   (free = 2*192 = 384)
  X[q, p, r, j, row] = bf16(ug[row, 1600*(2p+j) + 16*q + r]), q in
  [0,128) (115 live + 13 pad: 115-partition DMA measured 3.3x slower,
  but the matmul contracts over K=115 only).  16 matmuls per pair
  accumulate into psum_p[128, 384].  free=384 matters: the 128-row
  LDWEIGHTS (~150 ns) hides under a 384-cycle moving pass but NOT under
  192 (measured: free=192 makes the PE weight-load-bound, 64x133 ns).
  Epilogue per p: DVE copies psum[0:100, :] to SBUF, [100, 1536B] DMAs
  out; the host transposes [s, row] -> [row, s] during unshard (free).

Perf notes (measured via NTFF profiles):
  - Each of the 16 DMA queues moves ~26 GB/s independent of descriptor
    size; 16 queues saturate ~416 GB/s from a single HWDGE ring.  The two
    rings share the queues in coarse per-transfer bursts (NOT round-robin
    per descriptor), so interleaving consumption-ordered chunks across
    rings is unreliable; instead ALL input rides the scalar ring in strict
    consumption order (w, then (p, r-quad) chunks) and outputs ride the
    sync ring, where their fin-semaphore stalls can't block input.
  - Descriptor generation costs ~10 ns/desc on the issuing sequencer
    (~0.9 us per 128-partition transfer) and a transfer's descriptors
    reach the queues ~1.9 us after generation; 9 input transfers x 0.9 us
    of generation pipelines ahead of ~8.6 us of execution.
  - ~21 dummy matmuls on a memset tile bridge the HAM clock-gate warmup:
    the gate opens after ~4.3 us of CONTINUOUS PE activity (1.2 ->
    2.4 GHz); a gap re-gates it, and too many warms serialize ahead of
    the real matmuls.
"""

import numpy as np
import ml_dtypes

import concourse.bacc as bacc
import concourse.mybir as mybir
import concourse.tile as tile
from concourse.bass_utils import run_bass_kernel_spmd

# ---- problem constants (hardcoded per contract) ----
NINPUT, NDET, NTICK = 32, 48, 6400
NS = 16                    # downsample factor
S = NTICK // NS            # 400 output ticks
LIGHT_TICK = 0.1
CONV_TICKS = 990
NCORES = 8
N_PER_CORE = NINPUT // NCORES      # 4
ROWS = N_PER_CORE * NDET           # 192 rows per core
J = 15                             # q-steps of history (taps delta >= -16*J)
HALO = J
PAD = NS * HALO                    # 240 zero ticks prepended
TPAD = NTICK + PAD                 # 6640
STILE = 100                        # s-values per output tile
NST = S // STILE                   # 4
QW = STILE + HALO                  # 115 live q rows per tile
QP = 128                           # padded q partitions (DMA speed)
DMAX = NS * J                      # 240
N_WARM = 21                        # dummy matmuls to lift the HAM clock gate
WCOL = 128                         # weight columns (128 enables FWL)
NP = NST // 2                      # 2 pairs of s-tiles
PAIRW = 2 * ROWS                   # 384: free width of one pair matmul
XFREE = NP * NS * PAIRW            # 12288
NQUAD = 4                          # r-quads per pair: x chunk granularity
TALLOC = NS * STILE * (NST - 1) + NS * QP + NS   # 6848: strided-view extent

BF16 = ml_dtypes.bfloat16
E3M4 = ml_dtypes.float8_e3m4


def _build_taps(singlet_fraction_logit, log_tau_s, log_tau_t,
                light_oscillation_period, light_response_time):
    """c16[delta] for delta in [-DMAX, 15], float64."""
    dt = float(LIGHT_TICK)
    tt = np.arange(CONV_TICKS, dtype=np.float64)
    sf = 1.0 / (1.0 + np.exp(-float(singlet_fraction_logit)))
    tau_s = 10.0 ** float(log_tau_s)
    tau_t = 10.0 ** float(log_tau_t)
    per = float(light_oscillation_period)
    rt = float(light_response_time)
    p1 = sf * np.exp(-tt * dt / tau_s) * (1.0 - np.exp(-dt / tau_s))
    p3 = (1.0 - sf) * np.exp(-tt * dt / tau_t) * (1.0 - np.exp(-dt / tau_t))
    scint = p1 + p3
    t = tt * dt
    imp = np.exp(-t / rt) * np.sin(t / per)
    imp = imp / (per * rt * rt) * (per * per + rt * rt) * dt
    c = np.convolve(scint, imp)          # length 2*990-1 = 1979
    deltas = np.arange(-DMAX, 16)
    c16 = np.zeros(len(deltas), dtype=np.float64)
    for i, d in enumerate(deltas):
        ks = np.arange(max(0, d), 16)
        c16[i] = c[ks - d].sum()
    return c16                            # index i -> delta = i - DMAX


def _build_weights(c16):
    """W[q_rel, r, s_rel] float32 (QP rows, WCOL cols, zero-padded)."""
    w = np.zeros((QP, NS, WCOL), dtype=np.float64)
    q_rel = np.arange(QP)[:, None, None]
    r = np.arange(NS)[None, :, None]
    s_rel = np.arange(WCOL)[None, None, :]
    delta = 16 * (q_rel - HALO - s_rel) + r
    mask = ((delta >= -DMAX) & (delta <= 15) & (q_rel < QW)
            & (s_rel < STILE))
    w[mask] = c16[(delta + DMAX)[mask]]
    return np.ascontiguousarray(w, dtype=np.float32)


_PROGRAM = None


def _build_program():
    global _PROGRAM
    if _PROGRAM is not None:
        return _PROGRAM
    nc = bacc.Bacc("TRN2", target_bir_lowering=False, debug=False,
                   num_devices=NCORES)
    f32 = mybir.dt.float32
    bf16 = mybir.dt.bfloat16
    fp8 = mybir.dt.float8e3
    x_d = nc.dram_tensor("x", [QP, XFREE], bf16, kind="ExternalInput")
    w_d = nc.dram_tensor("w", [QP, NS * WCOL], fp8, kind="ExternalInput")
    o_d = nc.dram_tensor("out", [STILE, NP * PAIRW], f32,
                         kind="ExternalOutput")

    CH = XFREE // (NP * NQUAD)       # 1536: one (p, r-quad) chunk
    RQ = NS // NQUAD                 # 4 r per quad

    with tile.TileContext(nc) as tc:
        with (
            tc.tile_pool(name="const", bufs=1) as cpool,
            tc.tile_pool(name="x", bufs=1) as xpool,
            tc.tile_pool(name="fin", bufs=1) as fpool,
            tc.tile_pool(name="ps", bufs=1, space="PSUM") as pspool,
            tc.tile_pool(name="warm", bufs=1, space="PSUM") as wpool,
        ):
            # ALL input on the scalar ring in strict consumption order.
            # Dual-ring splits measured 1.9us SLOWER at both coarse and
            # per-quad granularity: the queues serve rings in bursts that
            # break consumption order.  All 128 q rows ship (115-partition
            # transfers measured 3.3x slower on HW!); the matmul contracts
            # over K=QW=115 only.  Outputs ride the idle sync ring.
            w_sb = cpool.tile([QP, NS * WCOL], fp8, tag="w")
            nc.scalar.dma_start(w_sb[:], w_d[:])

            x_sb = xpool.tile([QP, NP, NS, 2, ROWS], bf16, tag="x")
            x_flat = x_sb[:].rearrange("q p r j row -> q (p r j row)")
            for p in range(NP):
                for qd in range(NQUAD):
                    lo = (p * NS + qd * RQ) * PAIRW
                    if p == NP - 1 and qd == NQUAD - 1:
                        h = CH // 4
                        for i in range(4):
                            nc.scalar.dma_start(
                                x_flat[:, lo + i * h:lo + (i + 1) * h],
                                x_d[:, lo + i * h:lo + (i + 1) * h])
                    else:
                        nc.scalar.dma_start(x_flat[:, lo:lo + CH],
                                            x_d[:, lo:lo + CH])

            # PE warm-up: dummy bf16 matmuls on a memset tile (no DMA
            # dependency) keep TensorE busy so the HAM clock gate opens
            # (1.2 -> 2.4 GHz) just as the first chunks land.
            warm_w = cpool.tile([128, 256], bf16, tag="warmw")
            nc.vector.memset(warm_w[:], 1.0)
            ps_warm = wpool.tile([128, 256], f32, tag="warm")
            for _ in range(N_WARM):
                nc.tensor.matmul(ps_warm[:], warm_w[:, 0:128], warm_w[:],
                                 start=True, stop=True)

            for p in range(NP):
                ps = pspool.tile([WCOL, PAIRW], f32, tag=f"ps{p}")
                for r in range(NS):
                    nc.tensor.matmul(
                        ps[:], w_sb[0:QW, r * WCOL:(r + 1) * WCOL],
                        x_sb[0:QW, p, r, :, :],
                        start=(r == 0), stop=(r == NS - 1),
                    )
                fin_p = fpool.tile([STILE, PAIRW], f32, tag=f"fin{p}")
                nc.vector.tensor_copy(fin_p[:], ps[0:STILE, :])
                # outputs on the sync ring: their fin-semaphore stalls
                # must not head-of-line-block the input stream
                nc.sync.dma_start(o_d[:, p * PAIRW:(p + 1) * PAIRW],
                                  fin_p[:])

    nc.compile()
    _PROGRAM = nc
    return nc


def _prepare_inputs(timing_dist, singlet_fraction_logit, log_tau_s, log_tau_t,
                    light_oscillation_period, light_response_time, light_gain):
    u = np.ascontiguousarray(np.asarray(timing_dist, dtype=np.float32))
    assert u.shape == (NINPUT, NDET, NTICK)
    gain = np.asarray(light_gain, dtype=np.float32).reshape(NDET)

    c16 = _build_taps(singlet_fraction_logit, log_tau_s, log_tau_t,
                      light_oscillation_period, light_response_time)
    w = (_build_weights(c16).reshape(QP, NS * WCOL) * 16.0).astype(E3M4)

    ug = u * gain[None, :, None] * (1.0 / 16.0)   # gain folded in; /16
    # compensates the x16 weight scale (both exact powers of two)

    in_maps = []
    for c in range(NCORES):
        shard = ug[c * N_PER_CORE:(c + 1) * N_PER_CORE].reshape(ROWS, NTICK)
        up = np.zeros((ROWS, TALLOC), dtype=BF16)
        up[:, PAD:TPAD] = shard.astype(BF16)
        # polyphase relayout: x[q, p, r, j, row] =
        #     up[row, 1600*(2p+j) + 16*q + r]
        xs = np.lib.stride_tricks.as_strided(
            up,
            shape=(QP, NP, NS, 2, ROWS),
            strides=(NS * 2, 2 * NS * STILE * 2, 2, NS * STILE * 2,
                     up.strides[0]),
        )
        x = np.ascontiguousarray(xs).reshape(QP, XFREE)
        in_maps.append({"x": x, "w": w})
    return in_maps


def _run(in_maps, trace=False):
    nc = _build_program()
    res = run_bass_kernel_spmd(nc, in_maps, core_ids=list(range(NCORES)),
                               trace=trace)
    outs = []
    for c in range(NCORES):
        o = res.results[c]["out"].reshape(STILE, NP, 2, ROWS)
        # o[sr, p, j, row] = out[row, (2p+j)*100 + sr]
        outs.append(np.ascontiguousarray(o.transpose(3, 1, 2, 0))
                    .reshape(N_PER_CORE, NDET, S))
    full = np.concatenate(outs, axis=0).astype(np.float32, copy=False)
    return full, res


def kernel(timing_dist, singlet_fraction_logit, log_tau_s, log_tau_t,
           light_oscillation_period, light_response_time, light_gain):
    in_maps = _prepare_inputs(
        timing_dist, singlet_fraction_logit, log_tau_s, log_tau_t,
        light_oscillation_period, light_response_time, light_gain)
    full, _ = _run(in_maps, trace=False)
    return full


# revision 23
# speedup vs baseline: 1.1556x; 1.1556x over previous
"""BatchedLightSimulation Trainium2 kernel.

Math: the two causal convolutions (scintillation 990 taps, SiPM impulse 990
taps) compose into one 1979-tap causal filter c.  Folding the sum-by-16
downsample in gives

    out[row, s] = sum_delta c16[delta] * ug[row, 16*s + delta]

with c16[delta] = sum_{k=max(0,delta)}^{15} c[k - delta] and
ug[row, t] = gain[row] * u[row, t] (gain folded into the input on the
host).  c decays like exp(-l/15.3) so c16 truncated to delta >= -240 is
exact at fp32 precision.

Precision: x in single bf16 (no hi/lo split), W in fp8 e3m4 scaled by
16 (exact power of two, compensated by shipping x/16 -- bf16 absorbs it
in the exponent, so no epilogue scaling).  Measured end-to-end vs the
fp64 reference on the actual inputs: absmax-rel 4.34e-3, per-element
max 7.7e-3 -- inside the 2e-2 gate with >2.5x margin on either formula.
(x in fp8 was rejected: e3m4 x measures 9.4e-3 absmax / 1.7e-2
per-element -- too close to the gate.)

Device mapping (per core, 4 ninputs = 192 (n,d) rows):
  polyphase m = 16q + r; s tiled 4x100, tiles paired (p=0: s-tiles 0,1;
  p=1: s-tiles 2,3).  The banded weight is Toeplitz in (q_rel, s_rel), so
  ONE [128,128] weight slice per r serves every s-tile, and a pair's two
  tiles ride one matmul in the free dim:
      W_r.T[128q, 128s] @ X[q, p, r, :, :]   (free = 2*192 = 384)
  X[q, p, r, j, row] = bf16(ug[row, 1600*(2p+j) + 16*q + r]), q in
  [0,128) (115 live + 13 pad: 115-partition DMA measured 3.3x slower,
  but the matmul contracts over K=115 only).  16 matmuls per pair
  accumulate into psum_p[128, 384].  free=384 matters: the 128-row
  LDWEIGHTS (~150 ns) hides under a 384-cycle moving pass but NOT under
  192 (measured: free=192 makes the PE weight-load-bound, 64x133 ns).
  Epilogue per p: DVE copies psum[0:100, :] to SBUF, [100, 1536B] DMAs
  out; the host transposes [s, row] -> [row, s] during unshard (free).

Perf notes (measured via NTFF profiles):
  - Each of the 16 DMA queues moves ~26 GB/s independent of descriptor
    size; 16 queues saturate ~416 GB/s from a single HWDGE ring.  The two
    rings share the queues in coarse per-transfer bursts (NOT round-robin
    per descriptor), so interleaving consumption-ordered chunks across
    rings is unreliable; instead ALL input rides the scalar ring in strict
    consumption order (w, then (p, r-quad) chunks) and outputs ride the
    sync ring, where their fin-semaphore stalls can't block input.
  - Descriptor generation costs ~10 ns/desc on the issuing sequencer
    (~0.9 us per 128-partition transfer) and a transfer's descriptors
    reach the queues ~1.9 us after generation; 9 input transfers x 0.9 us
    of generation pipelines ahead of ~8.6 us of execution.
  - ~21 dummy matmuls on a memset tile bridge the HAM clock-gate warmup:
    the gate opens after ~4.3 us of CONTINUOUS PE activity (1.2 ->
    2.4 GHz); a gap re-gates it, and too many warms serialize ahead of
    the real matmuls.
"""

import numpy as np
import ml_dtypes

import concourse.bacc as bacc
import concourse.mybir as mybir
import concourse.tile as tile
from concourse.bass_utils import run_bass_kernel_spmd

# ---- problem constants (hardcoded per contract) ----
NINPUT, NDET, NTICK = 32, 48, 6400
NS = 16                    # downsample factor
S = NTICK // NS            # 400 output ticks
LIGHT_TICK = 0.1
CONV_TICKS = 990
NCORES = 8
N_PER_CORE = NINPUT // NCORES      # 4
ROWS = N_PER_CORE * NDET           # 192 rows per core
J = 15                             # q-steps of history (taps delta >= -16*J)
HALO = J
PAD = NS * HALO                    # 240 zero ticks prepended
TPAD = NTICK + PAD                 # 6640
STILE = 100                        # s-values per output tile
NST = S // STILE                   # 4
QW = STILE + HALO                  # 115 live q rows per tile
QP = 128                           # padded q partitions (DMA speed)
DMAX = NS * J                      # 240
N_WARM = 21                        # dummy matmuls to lift the HAM clock gate
WCOL = 128                         # weight columns (128 enables FWL)
NP = NST // 2                      # 2 pairs of s-tiles
PAIRW = 2 * ROWS                   # 384: free width of one pair matmul
XFREE = NP * NS * PAIRW            # 12288
NQUAD = 4                          # r-quads per pair: x chunk granularity
TALLOC = NS * STILE * (NST - 1) + NS * QP + NS   # 6848: strided-view extent

BF16 = ml_dtypes.bfloat16
E3M4 = ml_dtypes.float8_e3m4


def _build_taps(singlet_fraction_logit, log_tau_s, log_tau_t,
                light_oscillation_period, light_response_time):
    """c16[delta] for delta in [-DMAX, 15], float64."""
    dt = float(LIGHT_TICK)
    tt = np.arange(CONV_TICKS, dtype=np.float64)
    sf = 1.0 / (1.0 + np.exp(-float(singlet_fraction_logit)))
    tau_s = 10.0 ** float(log_tau_s)
    tau_t = 10.0 ** float(log_tau_t)
    per = float(light_oscillation_period)
    rt = float(light_response_time)
    p1 = sf * np.exp(-tt * dt / tau_s) * (1.0 - np.exp(-dt / tau_s))
    p3 = (1.0 - sf) * np.exp(-tt * dt / tau_t) * (1.0 - np.exp(-dt / tau_t))
    scint = p1 + p3
    t = tt * dt
    imp = np.exp(-t / rt) * np.sin(t / per)
    imp = imp / (per * rt * rt) * (per * per + rt * rt) * dt
    c = np.convolve(scint, imp)          # length 2*990-1 = 1979
    deltas = np.arange(-DMAX, 16)
    c16 = np.zeros(len(deltas), dtype=np.float64)
    for i, d in enumerate(deltas):
        ks = np.arange(max(0, d), 16)
        c16[i] = c[ks - d].sum()
    return c16                            # index i -> delta = i - DMAX


def _build_weights(c16):
    """W[q_rel, r, s_rel] float32 (QP rows, WCOL cols, zero-padded)."""
    w = np.zeros((QP, NS, WCOL), dtype=np.float64)
    q_rel = np.arange(QP)[:, None, None]
    r = np.arange(NS)[None, :, None]
    s_rel = np.arange(WCOL)[None, None, :]
    delta = 16 * (q_rel - HALO - s_rel) + r
    mask = ((delta >= -DMAX) & (delta <= 15) & (q_rel < QW)
            & (s_rel < STILE))
    w[mask] = c16[(delta + DMAX)[mask]]
    return np.ascontiguousarray(w, dtype=np.float32)


_PROGRAM = None


def _build_program():
    global _PROGRAM
    if _PROGRAM is not None:
        return _PROGRAM
    nc = bacc.Bacc("TRN2", target_bir_lowering=False, debug=False,
                   num_devices=NCORES)
    f32 = mybir.dt.float32
    bf16 = mybir.dt.bfloat16
    fp8 = mybir.dt.float8e3
    x_d = nc.dram_tensor("x", [QP, XFREE], bf16, kind="ExternalInput")
    w_d = nc.dram_tensor("w", [QP, NS * WCOL], fp8, kind="ExternalInput")
    o_d = nc.dram_tensor("out", [STILE, NP * PAIRW], f32,
                         kind="ExternalOutput")

    CH = XFREE // (NP * NQUAD)       # 1536: one (p, r-quad) chunk
    RQ = NS // NQUAD                 # 4 r per quad

    with tile.TileContext(nc) as tc:
        with (
            tc.tile_pool(name="const", bufs=1) as cpool,
            tc.tile_pool(name="x", bufs=1) as xpool,
            tc.tile_pool(name="fin", bufs=1) as fpool,
            tc.tile_pool(name="ps", bufs=1, space="PSUM") as pspool,
            tc.tile_pool(name="warm", bufs=1, space="PSUM") as wpool,
        ):
            # ALL input on the scalar ring in strict consumption order.
            # Dual-ring splits measured 1.9us SLOWER at both coarse and
            # per-quad granularity: the queues serve rings in bursts that
            # break consumption order.  All 128 q rows ship (115-partition
            # transfers measured 3.3x slower on HW!); the matmul contracts
            # over K=QW=115 only.  Outputs ride the idle sync ring.
            w_sb = cpool.tile([QP, NS * WCOL], fp8, tag="w")
            nc.scalar.dma_start(w_sb[:], w_d[:])

            x_sb = xpool.tile([QP, NP, NS, 2, ROWS], bf16, tag="x")
            x_flat = x_sb[:].rearrange("q p r j row -> q (p r j row)")
            for p in range(NP):
                for qd in range(NQUAD):
                    lo = (p * NS + qd * RQ) * PAIRW
                    if p == NP - 1 and qd == NQUAD - 1:
                        h = CH // 2
                        nc.scalar.dma_start(x_flat[:, lo:lo + h],
                                            x_d[:, lo:lo + h])
                        nc.scalar.dma_start(x_flat[:, lo + h:lo + CH],
                                            x_d[:, lo + h:lo + CH])
                    else:
                        nc.scalar.dma_start(x_flat[:, lo:lo + CH],
                                            x_d[:, lo:lo + CH])

            # PE warm-up: dummy bf16 matmuls on a memset tile (no DMA
            # dependency) keep TensorE busy so the HAM clock gate opens
            # (1.2 -> 2.4 GHz) just as the first chunks land.
            warm_w = cpool.tile([128, 256], bf16, tag="warmw")
            nc.vector.memset(warm_w[:], 1.0)
            ps_warm = wpool.tile([128, 256], f32, tag="warm")
            for _ in range(N_WARM):
                nc.tensor.matmul(ps_warm[:], warm_w[:, 0:128], warm_w[:],
                                 start=True, stop=True)

            for p in range(NP):
                ps = pspool.tile([WCOL, PAIRW], f32, tag=f"ps{p}")
                for r in range(NS):
                    nc.tensor.matmul(
                        ps[:], w_sb[0:QW, r * WCOL:(r + 1) * WCOL],
                        x_sb[0:QW, p, r, :, :],
                        start=(r == 0), stop=(r == NS - 1),
                    )
                fin_p = fpool.tile([STILE, PAIRW], f32, tag=f"fin{p}")
                nc.vector.tensor_copy(fin_p[:], ps[0:STILE, :])
                # outputs on the sync ring: their fin-semaphore stalls
                # must not head-of-line-block the input stream
                nc.sync.dma_start(o_d[:, p * PAIRW:(p + 1) * PAIRW],
                                  fin_p[:])

    nc.compile()
    _PROGRAM = nc
    return nc


def _prepare_inputs(timing_dist, singlet_fraction_logit, log_tau_s, log_tau_t,
                    light_oscillation_period, light_response_time, light_gain):
    u = np.ascontiguousarray(np.asarray(timing_dist, dtype=np.float32))
    assert u.shape == (NINPUT, NDET, NTICK)
    gain = np.asarray(light_gain, dtype=np.float32).reshape(NDET)

    c16 = _build_taps(singlet_fraction_logit, log_tau_s, log_tau_t,
                      light_oscillation_period, light_response_time)
    w = (_build_weights(c16).reshape(QP, NS * WCOL) * 16.0).astype(E3M4)

    ug = u * gain[None, :, None] * (1.0 / 16.0)   # gain folded in; /16
    # compensates the x16 weight scale (both exact powers of two)

    in_maps = []
    for c in range(NCORES):
        shard = ug[c * N_PER_CORE:(c + 1) * N_PER_CORE].reshape(ROWS, NTICK)
        up = np.zeros((ROWS, TALLOC), dtype=BF16)
        up[:, PAD:TPAD] = shard.astype(BF16)
        # polyphase relayout: x[q, p, r, j, row] =
        #     up[row, 1600*(2p+j) + 16*q + r]
        xs = np.lib.stride_tricks.as_strided(
            up,
            shape=(QP, NP, NS, 2, ROWS),
            strides=(NS * 2, 2 * NS * STILE * 2, 2, NS * STILE * 2,
                     up.strides[0]),
        )
        x = np.ascontiguousarray(xs).reshape(QP, XFREE)
        in_maps.append({"x": x, "w": w})
    return in_maps


def _run(in_maps, trace=False):
    nc = _build_program()
    res = run_bass_kernel_spmd(nc, in_maps, core_ids=list(range(NCORES)),
                               trace=trace)
    outs = []
    for c in range(NCORES):
        o = res.results[c]["out"].reshape(STILE, NP, 2, ROWS)
        # o[sr, p, j, row] = out[row, (2p+j)*100 + sr]
        outs.append(np.ascontiguousarray(o.transpose(3, 1, 2, 0))
                    .reshape(N_PER_CORE, NDET, S))
    full = np.concatenate(outs, axis=0).astype(np.float32, copy=False)
    return full, res


def kernel(timing_dist, singlet_fraction_logit, log_tau_s, log_tau_t,
           light_oscillation_period, light_response_time, light_gain):
    in_maps = _prepare_inputs(
        timing_dist, singlet_fraction_logit, log_tau_s, log_tau_t,
        light_oscillation_period, light_response_time, light_gain)
    full, _ = _run(in_maps, trace=False)
    return full
